# revision 4
# baseline (speedup 1.0000x reference)
"""Trainium2 Bass kernel for the segment distance-transform MSE loss.

Reference computes, for pred and gt polylines (2048 points -> 2047 segments):
    dist[g] = max_s keep_s * exp(-gamma * d2(s, g))   over a 128x128 grid
    loss = mean((dist_pred - dist_gt)^2)

Key identity: max_s exp(-gamma*d2) = exp(-gamma * min_s d2), so the device
only needs min-d2 per grid point.  Each segment's d2 decomposes into pure
quadratics in the grid coords:
    cand_s(g) = max(perp_s(g)^2, |g-c_s|^2 - r_s^2)     (exact inside slab,
                                                         safe overestimate out)
    E_e(g)    = |g - e|^2  for segment endpoints         (exact beyond caps)
    min_s d2 = min( min_s cand_s , min_e E_e )
All candidates are quadratic -> evaluated by TensorE matmuls over features
[dx^2, dx*dy, dy^2, dx, dy, 1] with dx,dy small integer pixel offsets (exact
under fp32r's 12-bit input truncation).  Coefficients are split hi/lo (K=12)
so fp32r matmuls are fp32-accurate at full speed.  VectorE does pairwise-max
and min reductions.  The grid is sharded 16 blocks (16x8 px) per core with
rank-matched assignment (cores get one block per size class, so the shared
SPMD program's per-slot shapes track the distribution, not the max); per-block
candidate lists are culled by a mathematically safe distance cut.
"""

import math
import numpy as np

GRID = 128
GAMMA = 200.0
DELTA = 2.0 / (GRID - 1)
BY, BX = 16, 8                  # block = 16 rows x 8 cols of pixels
NBY, NBX = GRID // BY, GRID // BX
NBLK = NBY * NBX                # 128 blocks
NCORES = 8
BPC = NBLK // NCORES            # 16 blocks per core
BIG = 1.0e6                     # padding / "dropped" distance^2
SLACK = math.log(1e6) / GAMMA   # exp slack for culling (rel err <= 1e-6)
PQUANT = 64                     # pair-count padding quantum
SQUANT = 128                    # single-count padding quantum

_compiled_cache = {}


# ----------------------------------------------------------------------------
# host-side geometry / coefficient construction
# ----------------------------------------------------------------------------

def _trunc12(x):
    """Round float32 array to 12 explicit mantissa bits (fp32r-exact)."""
    x = np.asarray(x, np.float64)
    m, e = np.frexp(x)
    return np.ldexp(np.round(m * 4096.0) / 4096.0, e).astype(np.float32)


def _block_geom():
    geoms = []
    for b in range(NBLK):
        brow, bcol = b // NBX, b % NBX
        X0 = (bcol * BX) * DELTA - 1.0
        Y0 = (brow * BY) * DELTA - 1.0
        # 2x2 sub-sample centers + covering radius of each sub-block
        sxs = [X0 + (BX / 4.0 - 0.5) * DELTA, X0 + (3 * BX / 4.0 - 0.5) * DELTA]
        sys_ = [Y0 + (BY / 4.0 - 0.5) * DELTA, Y0 + (3 * BY / 4.0 - 0.5) * DELTA]
        samples = [(sx, sy) for sy in sys_ for sx in sxs]
        hsub = math.hypot((BX / 4.0) * DELTA, (BY / 4.0) * DELTA)
        cx = X0 + (BX - 1) / 2.0 * DELTA
        cy = Y0 + (BY - 1) / 2.0 * DELTA
        hb = math.hypot((BX - 1) / 2.0 * DELTA, (BY - 1) / 2.0 * DELTA)
        geoms.append((X0, Y0, cx, cy, hb, samples, hsub))
    return geoms


_GEOMS = _block_geom()


def _features():
    """lhsT features [12, 128]: rows [F6; F6], F6 = [dx2, dxdy, dy2, dx, dy, 1]."""
    dx = np.arange(BX, dtype=np.float64)
    dy = np.arange(BY, dtype=np.float64)
    DXg, DYg = np.meshgrid(dx, dy)
    dxf = DXg.reshape(-1)                      # p = iy*BX + ix
    dyf = DYg.reshape(-1)
    F6 = np.stack([dxf * dxf, dxf * dyf, dyf * dyf, dxf, dyf,
                   np.ones_like(dxf)], axis=0)
    return np.concatenate([F6, F6], axis=0).astype(np.float32)  # [12, 128]


def _local_coeffs(quads, X0, Y0):
    """[n, 6] f64 quadratics over real coords -> [12, n] f32 hi/lo local rows."""
    a, b, c, d, e, f = (quads[:, i] for i in range(6))
    A2 = a * DELTA * DELTA
    B2 = b * DELTA * DELTA
    C2 = c * DELTA * DELTA
    D1 = (2 * a * X0 + b * Y0 + d) * DELTA
    E1 = (2 * c * Y0 + b * X0 + e) * DELTA
    F0 = a * X0 * X0 + b * X0 * Y0 + c * Y0 * Y0 + d * X0 + e * Y0 + f
    q = np.stack([A2, B2, C2, D1, E1, F0], axis=0)
    hi = _trunc12(q)
    lo = (q - hi.astype(np.float64)).astype(np.float32)
    return np.concatenate([hi, lo], axis=0)


def _transform_geometry(coords, is_pred):
    coords = np.asarray(coords, np.float32)
    kps = ((coords[:, :2] - np.float32(0.5)) * np.float32(2.0)).astype(np.float64)
    mask = (coords[:, 2] > 0.5) if is_pred else (coords[:, 2] != 0.0)
    keep = ~mask[:-1]
    A, B = kps[:-1], kps[1:]
    c = (A + B) / 2
    hv = (A - B) / 2
    r = np.hypot(hv[:, 0], hv[:, 1])
    rs = np.where(r > 0, r, 1)
    ux = np.where(r > 0, hv[:, 0] / rs, 1.0)
    uy = np.where(r > 0, hv[:, 1] / rs, 0.0)
    ep_act = np.zeros(len(kps), bool)
    ep_act[:-1] |= keep
    ep_act[1:] |= keep
    return dict(kps=kps, keep=keep, A=A, B=B, c=c, r=r,
                ux=ux, uy=uy, nx=-uy, ny=ux, ep_act=ep_act)


def _seg_point_dists(pts, geo):
    """pts [m, 2] -> distances [m, S] to all segments (f64)."""
    A, B = geo["A"], geo["B"]
    ab = B - A
    den = (ab * ab).sum(1)
    dens = np.where(den > 0, den, 1)
    t = ((pts[:, None, :] - A[None]) * ab[None]).sum(-1) / dens[None]
    t = np.clip(np.where(den[None] > 0, t, 0.0), 0.0, 1.0)
    proj = A[None] + t[..., None] * ab[None]
    dd = pts[:, None, :] - proj
    return np.hypot(dd[..., 0], dd[..., 1])


def _build_block_lists(geo, block):
    """Candidates for one (transform, block).

    Returns (pair_quads [np_, 2, 6], single_quads [ns, 6]) f64.
    """
    X0, Y0, cx, cy, hb, samples, hsub = _GEOMS[block]
    keep = geo["keep"]
    if not keep.any():
        return np.zeros((0, 2, 6)), np.zeros((0, 6))
    pts = np.asarray(samples)                   # [4, 2]
    dmat = _seg_point_dists(pts, geo)           # [4, S]
    dact = np.where(keep[None], dmat, np.inf)
    Dm = dact.min(1)                            # per-sample nearest active dist
    Rm = np.sqrt((Dm + hsub) ** 2 + SLACK) + hsub   # per-sample keep radius
    kept = keep & (dmat <= Rm[:, None]).any(0)

    c, r = geo["c"], geo["r"]
    mB = (cx - c[:, 0]) * geo["ux"] + (cy - c[:, 1]) * geo["uy"]
    inside = np.abs(mB) <= r - hb
    outside = np.abs(mB) >= r + hb
    pair_sel = kept & ~inside & ~outside
    singleQ_sel = kept & inside

    def q_perp(idx):
        nx, ny = geo["nx"][idx], geo["ny"][idx]
        cxs, cys = c[idx, 0], c[idx, 1]
        c0 = -(nx * cxs + ny * cys)
        return np.stack([nx * nx, 2 * nx * ny, ny * ny,
                         2 * nx * c0, 2 * ny * c0, c0 * c0], axis=1)

    def q_circ(px, py, rr2):
        one = np.ones_like(px)
        return np.stack([one, 0 * one, one, -2 * px, -2 * py,
                         px * px + py * py - rr2], axis=1)

    idx_p = np.nonzero(pair_sel)[0]
    pair_quads = np.zeros((len(idx_p), 2, 6))
    if len(idx_p):
        pair_quads[:, 0, :] = q_perp(idx_p)
        pair_quads[:, 1, :] = q_circ(c[idx_p, 0], c[idx_p, 1], r[idx_p] ** 2)

    idx_s = np.nonzero(singleQ_sel)[0]
    singles = [q_perp(idx_s)] if len(idx_s) else []

    kps, ep_act = geo["kps"], geo["ep_act"]
    dE = np.hypot(kps[:, None, 0] - pts[None, :, 0],
                  kps[:, None, 1] - pts[None, :, 1])   # [P, 4]
    ep_sel = ep_act & (dE <= Rm[None, :]).any(1)
    idx_e = np.nonzero(ep_sel)[0]
    if len(idx_e):
        singles.append(q_circ(kps[idx_e, 0], kps[idx_e, 1], np.zeros(len(idx_e))))
    single_quads = np.concatenate(singles, axis=0) if singles else np.zeros((0, 6))
    return pair_quads, single_quads


def _roundup(x, q):
    return max(q, ((x + q - 1) // q) * q)


def build_tables(pred_coords, gt_coords):
    """Build the execution plan + per-core coefficient tables.

    Returns (coef [NCORES, 12, C_total], plan) where plan has:
      core_blocks [NCORES, BPC], slots: list per bi of
      dict(off, C, t=[(NP, NS, poff, soff), (NP, NS, poff, soff)])
    """
    geos = [_transform_geometry(gt_coords, False),
            _transform_geometry(pred_coords, True)]
    lists = {}
    weight = np.zeros(NBLK, int)
    for b in range(NBLK):
        for t in range(2):
            pq, sq = _build_block_lists(geos[t], b)
            lists[(b, t)] = (pq, sq)
            weight[b] += 2 * len(pq) + len(sq)

    order = np.argsort(-weight)                 # desc by total columns
    core_blocks = np.zeros((NCORES, BPC), int)
    for g in range(BPC):                        # rank group g -> one block/core
        grp = order[g * NCORES:(g + 1) * NCORES]
        for cidx in range(NCORES):
            core_blocks[cidx, g] = grp[cidx]

    slots = []
    off = 0
    for g in range(BPC):
        grp = core_blocks[:, g]
        tinfo = []
        soff = off
        for t in range(2):
            NP = _roundup(max(len(lists[(b, t)][0]) for b in grp), PQUANT)
            NS = _roundup(max(len(lists[(b, t)][1]) for b in grp), SQUANT)
            tinfo.append((NP, NS))
        C = sum(2 * NP + NS for NP, NS in tinfo)
        t0 = (tinfo[0][0], tinfo[0][1], soff, soff + 2 * tinfo[0][0])
        base1 = soff + 2 * tinfo[0][0] + tinfo[0][1]
        t1 = (tinfo[1][0], tinfo[1][1], base1, base1 + 2 * tinfo[1][0])
        slots.append(dict(off=soff, C=C, t=[t0, t1]))
        off += C
    C_total = off

    coef = np.zeros((NCORES, 12, C_total), np.float32)
    coef[:, 5, :] = BIG                         # default pad: const hi = BIG
    for cidx in range(NCORES):
        for g in range(BPC):
            b = core_blocks[cidx, g]
            X0, Y0 = _GEOMS[b][0], _GEOMS[b][1]
            for t in range(2):
                NP, NS, poff, soff2 = slots[g]["t"][t]
                pq, sq = lists[(b, t)]
                quads = np.zeros((2 * NP + NS, 6))
                quads[:, 5] = BIG
                if len(pq):
                    quads[:2 * len(pq)] = pq.reshape(-1, 6)
                if len(sq):
                    quads[2 * NP:2 * NP + len(sq)] = sq
                coef[cidx, :, poff:poff + 2 * NP + NS] = \
                    _local_coeffs(quads, X0, Y0)
    plan = dict(core_blocks=core_blocks, slots=slots, C_total=C_total,
                key=tuple((s["t"][0][0], s["t"][0][1], s["t"][1][0], s["t"][1][1])
                          for s in slots))
    return coef, plan


# ----------------------------------------------------------------------------
# bass kernel build
# ----------------------------------------------------------------------------

def build_kernel(key, C_total, repeat=1):
    """key: per-slot (NP0, NS0, NP1, NS1) tuples; sizes baked statically."""
    import concourse.bacc as bacc
    import concourse.mybir as mybir
    import concourse.tile as tile

    f32, f32r = mybir.dt.float32, mybir.dt.float32r
    nc = bacc.Bacc(None, target_bir_lowering=False)
    feat_d = nc.dram_tensor("feat", [12, 128], f32, kind="ExternalInput")
    coef_d = nc.dram_tensor("coef", [12, C_total], f32, kind="ExternalInput")
    out_d = nc.dram_tensor("out", [128, BPC * 2], f32, kind="ExternalOutput")

    maxNP = max(max(k[0], k[2]) for k in key)
    maxparts = max(max(k[0] + (k[1] + 1023) // 1024,
                       k[2] + (k[3] + 1023) // 1024) for k in key)

    with tile.TileContext(nc) as tc:
        with (
            tc.tile_pool(name="feat", bufs=1) as featp,
            tc.tile_pool(name="coef", bufs=2) as coefp,
            tc.tile_pool(name="outsb", bufs=1) as outp,
            tc.tile_pool(name="parts", bufs=3) as partp,
            tc.tile_pool(name="psum", bufs=4, space="PSUM") as psum,
        ):
            feat = featp.tile([12, 128], f32r)
            nc.gpsimd.dma_start(feat[:], feat_d[:].bitcast(f32r))
            outsb = outp.tile([128, BPC * 2], f32)

            def mm_fill(ptile, cf, cf_off, ncols):
                for o in range(0, ncols, 512):
                    n = min(512, ncols - o)
                    nc.tensor.matmul(ptile[:, o:o + n], feat[:],
                                     cf[:, cf_off + o:cf_off + o + n],
                                     start=True, stop=True)

            def body(_iv=None):
                offs = [0]
                for k in key:
                    offs.append(offs[-1] + 2 * k[0] + k[1] + 2 * k[2] + k[3])
                for g in range(BPC):
                    NP0, NS0, NP1, NS1 = key[g]
                    cf = coefp.tile([12, offs[g + 1] - offs[g]], f32r, tag="cf")
                    nc.gpsimd.dma_start(
                        cf[:], coef_d[:, offs[g]:offs[g + 1]].bitcast(f32r))
                    lofs = 0
                    for t, (NP, NS) in enumerate(((NP0, NS0), (NP1, NS1))):
                        nparts = NP + (NS + 1023) // 1024
                        parts = partp.tile([128, maxparts], f32, tag="parts")
                        # pairs: matmul -> pairwise max -> parts[:, :NP]
                        for pc in range(0, NP, 512):
                            npair = min(512, NP - pc)
                            pt = psum.tile([128, 1024], f32, tag="ps")
                            mm_fill(pt, cf, lofs, 2 * npair)
                            nc.vector.tensor_reduce(
                                parts[:, pc:pc + npair],
                                pt[:, 0:2 * npair].rearrange(
                                    "p (n two) -> p n two", two=2),
                                axis=mybir.AxisListType.X, op=mybir.AluOpType.max)
                            lofs += 2 * npair
                        # singles: matmul chunks -> min -> parts[:, NP + j]
                        nsingle_units = (NS + 1023) // 1024
                        for j in range(nsingle_units):
                            ncols = min(1024, NS - j * 1024)
                            st = psum.tile([128, 1024], f32, tag="ps")
                            mm_fill(st, cf, lofs, ncols)
                            nc.vector.tensor_reduce(
                                parts[:, NP + j:NP + j + 1], st[:, 0:ncols],
                                axis=mybir.AxisListType.X, op=mybir.AluOpType.min)
                            lofs += ncols
                        nc.vector.tensor_reduce(
                            outsb[:, g * 2 + t:g * 2 + t + 1],
                            parts[:, 0:nparts],
                            axis=mybir.AxisListType.X, op=mybir.AluOpType.min)

            if repeat == 1:
                body()
            else:
                with tc.For_i(0, repeat, 1) as iv:
                    body(iv)
            nc.gpsimd.dma_start(out_d[:], outsb[:])
    nc.compile()
    return nc


def get_runner(key, C_total, repeat=1):
    ck = (key, C_total, repeat)
    if ck not in _compiled_cache:
        nc = build_kernel(key, C_total, repeat)
        _compiled_cache[ck] = _SpmdRunner(nc, NCORES)
    return _compiled_cache[ck]


# ----------------------------------------------------------------------------
# jit-once SPMD runner (axon PJRT path)
# ----------------------------------------------------------------------------

class _SpmdRunner:
    def __init__(self, nc, n_cores):
        import jax
        import concourse.mybir as mybir
        from jax.sharding import Mesh, PartitionSpec
        from jax.experimental.shard_map import shard_map
        from concourse.bass2jax import (_bass_exec_p, install_neuronx_cc_hook,
                                        partition_id_tensor)
        self.jax = jax
        install_neuronx_cc_hook()
        self.nc = nc
        self.n_cores = n_cores
        partition_name = (nc.partition_id_tensor.name
                          if nc.partition_id_tensor else None)
        in_names, out_names, out_avals, zero_outs = [], [], [], []
        for alloc in nc.m.functions[0].allocations:
            if not isinstance(alloc, mybir.MemoryLocationSet):
                continue
            name = alloc.memorylocations[0].name
            if alloc.kind == "ExternalInput":
                if name != partition_name:
                    in_names.append(name)
            elif alloc.kind == "ExternalOutput":
                out_names.append(name)
                shape = tuple(alloc.tensor_shape)
                dtype = mybir.dt.np(alloc.dtype)
                out_avals.append(jax.core.ShapedArray(shape, dtype))
                zero_outs.append(np.zeros(shape, dtype))
        self.in_names = in_names
        self.out_names = out_names
        self.zero_outs = zero_outs
        n_params, n_outs = len(in_names), len(out_names)
        all_in = in_names + out_names + ([partition_name] if partition_name else [])

        def _body(*args):
            operands = list(args)
            if partition_name is not None:
                operands.append(partition_id_tensor())
            outs = _bass_exec_p.bind(
                *operands, out_avals=tuple(out_avals), in_names=tuple(all_in),
                out_names=tuple(out_names), lowering_input_output_aliases=(),
                sim_require_finite=True, sim_require_nnan=True, nc=nc)
            return tuple(outs)

        devices = jax.devices()[:n_cores]
        self.mesh = Mesh(np.asarray(devices), ("core",))
        self.fn = jax.jit(
            shard_map(_body, mesh=self.mesh,
                      in_specs=(PartitionSpec("core"),) * (n_params + n_outs),
                      out_specs=(PartitionSpec("core"),) * n_outs,
                      check_rep=False),
            donate_argnums=tuple(range(n_params, n_params + n_outs)),
            keep_unused=True)
        self.sharding = jax.sharding.NamedSharding(self.mesh, PartitionSpec("core"))

    def put_inputs(self, in_maps):
        return [self.jax.device_put(
                    np.concatenate([np.asarray(m[n]) for m in in_maps], axis=0),
                    self.sharding)
                for n in self.in_names]

    def run(self, dev_in):
        zo = [self.jax.device_put(np.concatenate([z] * self.n_cores, axis=0),
                                  self.sharding) for z in self.zero_outs]
        outs = self.fn(*dev_in, *zo)
        self.jax.block_until_ready(outs)
        results = []
        for c in range(self.n_cores):
            m = {}
            for i, name in enumerate(self.out_names):
                arr = np.asarray(outs[i])
                per = arr.shape[0] // self.n_cores
                m[name] = arr[c * per:(c + 1) * per]
            results.append(m)
        return results


# ----------------------------------------------------------------------------
# entry point
# ----------------------------------------------------------------------------

def _finish(d2_gt, d2_pred):
    beta_g = np.exp(-GAMMA * d2_gt.astype(np.float64))
    beta_p = np.exp(-GAMMA * d2_pred.astype(np.float64))
    return np.array(np.mean((beta_p - beta_g) ** 2), dtype=np.float32)


def _assemble(results, core_blocks):
    d2 = np.zeros((2, GRID, GRID), np.float32)
    for cidx in range(NCORES):
        out = results[cidx]["out"]          # [128, BPC*2]
        for g in range(BPC):
            b = core_blocks[cidx, g]
            brow, bcol = b // NBX, b % NBX
            for t in range(2):
                d2[t, brow * BY:(brow + 1) * BY,
                   bcol * BX:(bcol + 1) * BX] = \
                    out[:, g * 2 + t].reshape(BY, BX)
    return d2


def kernel(pred_coords, gt_coords):
    coef, plan = build_tables(pred_coords, gt_coords)
    feat = _features()
    runner = get_runner(plan["key"], plan["C_total"])
    in_maps = [{"feat": feat, "coef": coef[c]} for c in range(NCORES)]
    dev_in = runner.put_inputs(in_maps)
    results = runner.run(dev_in)
    d2 = _assemble(results, plan["core_blocks"])
    return _finish(d2[0], d2[1])


# revision 21
# speedup vs baseline: 1.4007x; 1.4007x over previous
"""Trainium2 Bass kernel for the segment distance-transform MSE loss.

Reference computes, for pred and gt polylines (2048 points -> 2047 segments):
    dist[g] = max_s keep_s * exp(-gamma * d2(s, g))   over a 128x128 grid
    loss = mean((dist_pred - dist_gt)^2)

Key identity: max_s exp(-gamma*d2) = exp(-gamma * min_s d2), so the device
only needs min-d2 per grid point.  Each segment's d2 decomposes into pure
quadratics in the grid coords:
    cand_s(g) = max(perp_s(g)^2, |g-c_s|^2 - r_s^2)     (exact inside slab,
                                                         safe overestimate out)
    E_e(g)    = |g - e|^2  for segment endpoints         (exact beyond caps)
    min_s d2 = min( min_s cand_s , min_e E_e )
All candidates are quadratic -> evaluated by TensorE matmuls over features
[dx^2, dx*dy, dy^2, dx, dy, 1] with dx,dy small integer pixel offsets (exact
under fp32r's 12-bit input truncation).  Coefficients are split hi/lo (K=12)
so fp32r matmuls are fp32-accurate at full speed.  VectorE does pairwise-max
and min reductions.  The grid is sharded 16 blocks (16x8 px) per core with
rank-matched assignment (cores get one block per size class, so the shared
SPMD program's per-slot shapes track the distribution, not the max); per-block
candidate lists are culled by a mathematically safe distance cut.
"""

import math
import numpy as np

GRID = 128
GAMMA = 200.0
DELTA = 2.0 / (GRID - 1)
BY, BX = 16, 8                  # block = 16 rows x 8 cols of pixels
NBY, NBX = GRID // BY, GRID // BX
NBLK = NBY * NBX                # 128 blocks
NCORES = 8
BPC = NBLK // NCORES            # 16 blocks per core
BIG = 1.0e6                     # padding / "dropped" distance^2
SLACK = math.log(1e5) / GAMMA   # exp slack for culling (rel err <= 1e-5)
PQUANT = 32                     # pair-count padding quantum
SQUANT = 64                     # single-count padding quantum

_compiled_cache = {}


# ----------------------------------------------------------------------------
# host-side geometry / coefficient construction
# ----------------------------------------------------------------------------

def _trunc12(x):
    """Round float32 array to 12 explicit mantissa bits (fp32r-exact)."""
    x = np.asarray(x, np.float64)
    m, e = np.frexp(x)
    return np.ldexp(np.round(m * 4096.0) / 4096.0, e).astype(np.float32)


def _block_geom():
    geoms = []
    for b in range(NBLK):
        brow, bcol = b // NBX, b % NBX
        X0 = (bcol * BX) * DELTA - 1.0
        Y0 = (brow * BY) * DELTA - 1.0
        # 2x4 sub-sample centers (4x4 px sub-blocks) + covering radius
        sxs = [X0 + (sx * 4 + 1.5) * DELTA for sx in range(BX // 4)]
        sys_ = [Y0 + (sy * 4 + 1.5) * DELTA for sy in range(BY // 4)]
        samples = [(sx, sy) for sy in sys_ for sx in sxs]
        hsub = math.hypot(1.5 * DELTA, 1.5 * DELTA)
        cx = X0 + (BX - 1) / 2.0 * DELTA
        cy = Y0 + (BY - 1) / 2.0 * DELTA
        hb = math.hypot((BX - 1) / 2.0 * DELTA, (BY - 1) / 2.0 * DELTA)
        geoms.append((X0, Y0, cx, cy, hb, samples, hsub))
    return geoms


_GEOMS = _block_geom()


def _features():
    """lhsT features [12, 128]: rows [F6; F6], F6 = [dx2, dxdy, dy2, dx, dy, 1]."""
    dx = np.arange(BX, dtype=np.float64)
    dy = np.arange(BY, dtype=np.float64)
    DXg, DYg = np.meshgrid(dx, dy)
    dxf = DXg.reshape(-1)                      # p = iy*BX + ix
    dyf = DYg.reshape(-1)
    F6 = np.stack([dxf * dxf, dxf * dyf, dyf * dyf, dxf, dyf,
                   np.ones_like(dxf)], axis=0)
    return np.concatenate([F6, F6], axis=0).astype(np.float32)  # [12, 128]


def _local_coeffs(quads, X0, Y0):
    """[n, 6] f64 quadratics over real coords -> [12, n] f32 hi/lo local rows."""
    a, b, c, d, e, f = (quads[:, i] for i in range(6))
    A2 = a * DELTA * DELTA
    B2 = b * DELTA * DELTA
    C2 = c * DELTA * DELTA
    D1 = (2 * a * X0 + b * Y0 + d) * DELTA
    E1 = (2 * c * Y0 + b * X0 + e) * DELTA
    F0 = a * X0 * X0 + b * X0 * Y0 + c * Y0 * Y0 + d * X0 + e * Y0 + f
    q = np.stack([A2, B2, C2, D1, E1, F0], axis=0)
    hi = _trunc12(q)
    lo = (q - hi.astype(np.float64)).astype(np.float32)
    return np.concatenate([hi, lo], axis=0)


def _transform_geometry(coords, is_pred):
    coords = np.asarray(coords, np.float32)
    kps = ((coords[:, :2] - np.float32(0.5)) * np.float32(2.0)).astype(np.float64)
    mask = (coords[:, 2] > 0.5) if is_pred else (coords[:, 2] != 0.0)
    keep = ~mask[:-1]
    A, B = kps[:-1], kps[1:]
    c = (A + B) / 2
    hv = (A - B) / 2
    r = np.hypot(hv[:, 0], hv[:, 1])
    rs = np.where(r > 0, r, 1)
    ux = np.where(r > 0, hv[:, 0] / rs, 1.0)
    uy = np.where(r > 0, hv[:, 1] / rs, 0.0)
    ep_act = np.zeros(len(kps), bool)
    ep_act[:-1] |= keep
    ep_act[1:] |= keep
    return dict(kps=kps, keep=keep, A=A, B=B, c=c, r=r,
                ux=ux, uy=uy, nx=-uy, ny=ux, ep_act=ep_act)


def _seg_point_dists(pts, geo):
    """pts [m, 2] -> distances [m, S] to all segments (f64)."""
    A, B = geo["A"], geo["B"]
    ab = B - A
    den = (ab * ab).sum(1)
    dens = np.where(den > 0, den, 1)
    t = ((pts[:, None, :] - A[None]) * ab[None]).sum(-1) / dens[None]
    t = np.clip(np.where(den[None] > 0, t, 0.0), 0.0, 1.0)
    proj = A[None] + t[..., None] * ab[None]
    dd = pts[:, None, :] - proj
    return np.hypot(dd[..., 0], dd[..., 1])


def _build_block_lists(geo, block):
    """Candidates for one (transform, block).

    Returns (pair_quads [np_, 2, 6], single_quads [ns, 6]) f64.
    """
    X0, Y0, cx, cy, hb, samples, hsub = _GEOMS[block]
    keep = geo["keep"]
    if not keep.any():
        return np.zeros((0, 2, 6)), np.zeros((0, 6))
    pts = np.asarray(samples)                   # [m, 2]
    dmat = _seg_point_dists(pts, geo)           # [m, S]
    dact = np.where(keep[None], dmat, np.inf)
    Dm = dact.min(1)                            # per-sample nearest active dist
    Rm = np.sqrt((Dm + hsub) ** 2 + SLACK) + hsub   # per-sample keep radius
    kept = keep & (dmat <= Rm[:, None]).any(0)

    c, r = geo["c"], geo["r"]
    # per-sample axis coordinate m_i for each segment: [m, S]
    mS = ((pts[:, None, 0] - c[None, :, 0]) * geo["ux"][None]
          + (pts[:, None, 1] - c[None, :, 1]) * geo["uy"][None])
    inside = (np.abs(mS) <= (r - hsub)[None]).all(0)
    outside = ((mS >= (r + hsub)[None]).all(0)
               | (mS <= -(r + hsub)[None]).all(0))
    pair_sel = kept & ~inside & ~outside
    singleQ_sel = kept & inside
    # cap-side reachability (for endpoint wedge culling)
    reachA = (mS >= (r - hsub)[None]).any(0)    # block reaches beyond A end
    reachB = (mS <= -(r - hsub)[None]).any(0)   # ... beyond B end

    def q_perp(idx):
        nx, ny = geo["nx"][idx], geo["ny"][idx]
        cxs, cys = c[idx, 0], c[idx, 1]
        c0 = -(nx * cxs + ny * cys)
        return np.stack([nx * nx, 2 * nx * ny, ny * ny,
                         2 * nx * c0, 2 * ny * c0, c0 * c0], axis=1)

    def q_circ(px, py, rr2):
        one = np.ones_like(px)
        return np.stack([one, 0 * one, one, -2 * px, -2 * py,
                         px * px + py * py - rr2], axis=1)

    idx_p = np.nonzero(pair_sel)[0]
    pair_quads = np.zeros((len(idx_p), 2, 6))
    if len(idx_p):
        pair_quads[:, 0, :] = q_perp(idx_p)
        pair_quads[:, 1, :] = q_circ(c[idx_p, 0], c[idx_p, 1], r[idx_p] ** 2)

    idx_s = np.nonzero(singleQ_sel)[0]
    singles = [q_perp(idx_s)] if len(idx_s) else []

    kps = geo["kps"]
    # endpoint kps[i] is the A-end of segment i and the B-end of segment i-1;
    # it is only needed where the block reaches beyond that cap.
    npnt = len(kps)
    wedge = np.zeros(npnt, bool)
    wedge[:-1] |= kept & reachA                 # as A-end of segment i
    wedge[1:] |= kept & reachB                  # as B-end of segment i-1
    dE = np.hypot(kps[:, None, 0] - pts[None, :, 0],
                  kps[:, None, 1] - pts[None, :, 1])   # [P, m]
    ep_sel = wedge & (dE <= Rm[None, :]).any(1)
    idx_e = np.nonzero(ep_sel)[0]
    if len(idx_e):
        singles.append(q_circ(kps[idx_e, 0], kps[idx_e, 1], np.zeros(len(idx_e))))
    single_quads = np.concatenate(singles, axis=0) if singles else np.zeros((0, 6))
    return pair_quads, single_quads


def _roundup(x, q):
    return max(q, ((x + q - 1) // q) * q)


def build_tables(pred_coords, gt_coords):
    """Build the execution plan + per-core coefficient tables.

    Returns (coef [NCORES, 12, C_total], plan) where plan has:
      core_blocks [NCORES, BPC], slots: list per bi of
      dict(off, C, t=[(NP, NS, poff, soff), (NP, NS, poff, soff)])
    """
    geos = [_transform_geometry(gt_coords, False),
            _transform_geometry(pred_coords, True)]
    lists = {}
    weight = np.zeros(NBLK, int)
    for b in range(NBLK):
        for t in range(2):
            pq, sq = _build_block_lists(geos[t], b)
            lists[(b, t)] = (pq, sq)
            weight[b] += 2 * len(pq) + len(sq)

    order = np.argsort(-weight)                 # desc by total columns
    core_blocks = np.zeros((NCORES, BPC), int)
    for g in range(BPC):                        # rank group g -> one block/core
        grp = order[g * NCORES:(g + 1) * NCORES]
        for cidx in range(NCORES):
            core_blocks[cidx, g] = grp[cidx]

    slots = []
    off = 0
    for g in range(BPC):
        grp = core_blocks[:, g]
        tinfo = []
        soff = off
        for t in range(2):
            NP = _roundup(max(len(lists[(b, t)][0]) for b in grp), PQUANT)
            NS = _roundup(max(len(lists[(b, t)][1]) for b in grp), SQUANT)
            tinfo.append((NP, NS))
        C = sum(2 * NP + NS for NP, NS in tinfo)
        t0 = (tinfo[0][0], tinfo[0][1], soff, soff + 2 * tinfo[0][0])
        base1 = soff + 2 * tinfo[0][0] + tinfo[0][1]
        t1 = (tinfo[1][0], tinfo[1][1], base1, base1 + 2 * tinfo[1][0])
        slots.append(dict(off=soff, C=C, t=[t0, t1]))
        off += C
    C_total = off

    coef = np.zeros((NCORES, 12, C_total), np.float32)
    coef[:, 5, :] = BIG                         # default pad: const hi = BIG
    for cidx in range(NCORES):
        for g in range(BPC):
            b = core_blocks[cidx, g]
            X0, Y0 = _GEOMS[b][0], _GEOMS[b][1]
            for t in range(2):
                NP, NS, poff, soff2 = slots[g]["t"][t]
                pq, sq = lists[(b, t)]
                quads = np.zeros((2 * NP + NS, 6))
                quads[:, 5] = BIG
                if len(pq):
                    # pair layout: [Q cols (NP) | Q2 cols (NP)]
                    quads[:len(pq)] = pq[:, 0]
                    quads[NP:NP + len(pq)] = pq[:, 1]
                if len(sq):
                    quads[2 * NP:2 * NP + len(sq)] = sq
                coef[cidx, :, poff:poff + 2 * NP + NS] = \
                    _local_coeffs(quads, X0, Y0)
    plan = dict(core_blocks=core_blocks, slots=slots, C_total=C_total,
                key=tuple((s["t"][0][0], s["t"][0][1], s["t"][1][0], s["t"][1][1])
                          for s in slots))
    return coef, plan


# ----------------------------------------------------------------------------
# bass kernel build
# ----------------------------------------------------------------------------

def build_kernel(key, C_total, repeat=1):
    """key: per-slot (NP0, NS0, NP1, NS1) tuples; sizes baked statically."""
    import concourse.bacc as bacc
    import concourse.mybir as mybir
    import concourse.tile as tile

    f32, f32r = mybir.dt.float32, mybir.dt.float32r
    nc = bacc.Bacc(None, target_bir_lowering=False)
    feat_d = nc.dram_tensor("feat", [12, 128], f32, kind="ExternalInput")
    coef_d = nc.dram_tensor("coef", [12, C_total], f32, kind="ExternalInput")
    out_d = nc.dram_tensor("out", [128, BPC * 2], f32, kind="ExternalOutput")

    maxscr = 1024

    with tile.TileContext(nc) as tc:
        with (
            tc.tile_pool(name="feat", bufs=1) as featp,
            tc.tile_pool(name="coef", bufs=2) as coefp,
            tc.tile_pool(name="outsb", bufs=1) as outp,
            tc.tile_pool(name="scr", bufs=2) as scrp,
            tc.tile_pool(name="cpy", bufs=3) as cpyp,
            tc.tile_pool(name="acc", bufs=3) as accp,
            tc.tile_pool(name="ppsum", bufs=2, space="PSUM") as ppsum,
            tc.tile_pool(name="spsum", bufs=3, space="PSUM") as spsum,
        ):
            feat = featp.tile([12, 128], f32r)
            nc.gpsimd.dma_start(feat[:], feat_d[:].bitcast(f32r))
            outsb = outp.tile([128, BPC * 2], f32)

            def mm_fill(ptile, cf, cf_off, ncols):
                for o in range(0, ncols, 512):
                    n = min(512, ncols - o)
                    nc.tensor.matmul(ptile[:, o:o + n], feat[:],
                                     cf[:, cf_off + o:cf_off + o + n],
                                     start=True, stop=True)

            def body(_iv=None):
                offs = [0]
                for k in key:
                    offs.append(offs[-1] + 2 * k[0] + k[1] + 2 * k[2] + k[3])
                for g in range(BPC):
                    NP0, NS0, NP1, NS1 = key[g]
                    cf = coefp.tile([12, offs[g + 1] - offs[g]], f32r, tag="cf")
                    nc.gpsimd.dma_start(
                        cf[:], coef_d[:, offs[g]:offs[g + 1]].bitcast(f32r))
                    lofs = 0
                    for t, (NP, NS) in enumerate(((NP0, NS0), (NP1, NS1))):
                        u_s = (NS + 1023) // 1024
                        parts = scrp.tile([128, maxscr], f32, tag="parts")
                        # pairs [Q | Q2]: ScalarE bounces Q2 PSUM->SBUF, DVE
                        # computes max(Q, Q2copy) straight into parts
                        pbase = lofs
                        for pc in range(0, NP, 512):
                            npair = min(512, NP - pc)
                            ptA = ppsum.tile([128, 512], f32, tag="pp")
                            ptB = ppsum.tile([128, 512], f32, tag="pp")
                            mm_fill(ptA, cf, pbase + pc, npair)
                            mm_fill(ptB, cf, pbase + NP + pc, npair)
                            cb = cpyp.tile([128, 512], f32, tag="cpy")
                            nc.scalar.copy(cb[:, 0:npair], ptB[:, 0:npair])
                            nc.vector.tensor_tensor(
                                parts[:, pc:pc + npair], ptA[:, 0:npair],
                                cb[:, 0:npair], op=mybir.AluOpType.max)
                        lofs += 2 * NP
                        # singles: reduce-min straight from PSUM into parts
                        for j in range(u_s):
                            ncols = min(1024, NS - j * 1024)
                            st = spsum.tile([128, 1024], f32, tag="sp")
                            mm_fill(st, cf, lofs, ncols)
                            nc.vector.tensor_reduce(
                                parts[:, NP + j:NP + j + 1], st[:, 0:ncols],
                                axis=mybir.AxisListType.X,
                                op=mybir.AluOpType.min)
                            lofs += ncols
                        nc.vector.tensor_reduce(
                            outsb[:, g * 2 + t:g * 2 + t + 1],
                            parts[:, 0:NP + u_s],
                            axis=mybir.AxisListType.X, op=mybir.AluOpType.min)

            if repeat == 1:
                body()
            else:
                with tc.For_i(0, repeat, 1) as iv:
                    body(iv)
            nc.gpsimd.dma_start(out_d[:], outsb[:])
    nc.compile()
    return nc


def get_runner(key, C_total, repeat=1):
    ck = (key, C_total, repeat)
    if ck not in _compiled_cache:
        nc = build_kernel(key, C_total, repeat)
        _compiled_cache[ck] = _SpmdRunner(nc, NCORES)
    return _compiled_cache[ck]


# ----------------------------------------------------------------------------
# jit-once SPMD runner (axon PJRT path)
# ----------------------------------------------------------------------------

class _SpmdRunner:
    def __init__(self, nc, n_cores):
        import jax
        import concourse.mybir as mybir
        from jax.sharding import Mesh, PartitionSpec
        from jax.experimental.shard_map import shard_map
        from concourse.bass2jax import (_bass_exec_p, install_neuronx_cc_hook,
                                        partition_id_tensor)
        self.jax = jax
        install_neuronx_cc_hook()
        self.nc = nc
        self.n_cores = n_cores
        partition_name = (nc.partition_id_tensor.name
                          if nc.partition_id_tensor else None)
        in_names, out_names, out_avals, zero_outs = [], [], [], []
        for alloc in nc.m.functions[0].allocations:
            if not isinstance(alloc, mybir.MemoryLocationSet):
                continue
            name = alloc.memorylocations[0].name
            if alloc.kind == "ExternalInput":
                if name != partition_name:
                    in_names.append(name)
            elif alloc.kind == "ExternalOutput":
                out_names.append(name)
                shape = tuple(alloc.tensor_shape)
                dtype = mybir.dt.np(alloc.dtype)
                out_avals.append(jax.core.ShapedArray(shape, dtype))
                zero_outs.append(np.zeros(shape, dtype))
        self.in_names = in_names
        self.out_names = out_names
        self.zero_outs = zero_outs
        n_params, n_outs = len(in_names), len(out_names)
        all_in = in_names + out_names + ([partition_name] if partition_name else [])

        def _body(*args):
            operands = list(args)
            if partition_name is not None:
                operands.append(partition_id_tensor())
            outs = _bass_exec_p.bind(
                *operands, out_avals=tuple(out_avals), in_names=tuple(all_in),
                out_names=tuple(out_names), lowering_input_output_aliases=(),
                sim_require_finite=True, sim_require_nnan=True, nc=nc)
            return tuple(outs)

        devices = jax.devices()[:n_cores]
        self.mesh = Mesh(np.asarray(devices), ("core",))
        self.fn = jax.jit(
            shard_map(_body, mesh=self.mesh,
                      in_specs=(PartitionSpec("core"),) * (n_params + n_outs),
                      out_specs=(PartitionSpec("core"),) * n_outs,
                      check_rep=False),
            donate_argnums=tuple(range(n_params, n_params + n_outs)),
            keep_unused=True)
        self.sharding = jax.sharding.NamedSharding(self.mesh, PartitionSpec("core"))

    def put_inputs(self, in_maps):
        return [self.jax.device_put(
                    np.concatenate([np.asarray(m[n]) for m in in_maps], axis=0),
                    self.sharding)
                for n in self.in_names]

    def run(self, dev_in):
        zo = [self.jax.device_put(np.concatenate([z] * self.n_cores, axis=0),
                                  self.sharding) for z in self.zero_outs]
        outs = self.fn(*dev_in, *zo)
        self.jax.block_until_ready(outs)
        results = []
        for c in range(self.n_cores):
            m = {}
            for i, name in enumerate(self.out_names):
                arr = np.asarray(outs[i])
                per = arr.shape[0] // self.n_cores
                m[name] = arr[c * per:(c + 1) * per]
            results.append(m)
        return results


# ----------------------------------------------------------------------------
# entry point
# ----------------------------------------------------------------------------

def _finish(d2_gt, d2_pred):
    beta_g = np.exp(-GAMMA * d2_gt.astype(np.float64))
    beta_p = np.exp(-GAMMA * d2_pred.astype(np.float64))
    return np.array(np.mean((beta_p - beta_g) ** 2), dtype=np.float32)


def _assemble(results, core_blocks):
    d2 = np.zeros((2, GRID, GRID), np.float32)
    for cidx in range(NCORES):
        out = results[cidx]["out"]          # [128, BPC*2]
        for g in range(BPC):
            b = core_blocks[cidx, g]
            brow, bcol = b // NBX, b % NBX
            for t in range(2):
                d2[t, brow * BY:(brow + 1) * BY,
                   bcol * BX:(bcol + 1) * BX] = \
                    out[:, g * 2 + t].reshape(BY, BX)
    return d2


def kernel(pred_coords, gt_coords):
    coef, plan = build_tables(pred_coords, gt_coords)
    feat = _features()
    runner = get_runner(plan["key"], plan["C_total"])
    in_maps = [{"feat": feat, "coef": coef[c]} for c in range(NCORES)]
    dev_in = runner.put_inputs(in_maps)
    results = runner.run(dev_in)
    d2 = _assemble(results, plan["core_blocks"])
    return _finish(d2[0], d2[1])


# revision 26
# speedup vs baseline: 1.4484x; 1.0340x over previous
"""Trainium2 Bass kernel for the segment distance-transform MSE loss.

Reference computes, for pred and gt polylines (2048 points -> 2047 segments):
    dist[g] = max_s keep_s * exp(-gamma * d2(s, g))   over a 128x128 grid
    loss = mean((dist_pred - dist_gt)^2)

Key identity: max_s exp(-gamma*d2) = exp(-gamma * min_s d2), so the device
only needs min-d2 per grid point.  Each segment's d2 decomposes into pure
quadratics in the grid coords:
    cand_s(g) = max(perp_s(g)^2, |g-c_s|^2 - r_s^2)     (exact inside slab,
                                                         safe overestimate out)
    E_e(g)    = |g - e|^2  for segment endpoints         (exact beyond caps)
    min_s d2 = min( min_s cand_s , min_e E_e )
All candidates are quadratic -> evaluated by TensorE matmuls over features
[dx^2, dx*dy, dy^2, dx, dy, 1] with dx,dy small integer pixel offsets (exact
under fp32r's 12-bit input truncation).  Coefficients are split hi/lo (K=12)
so fp32r matmuls are fp32-accurate at full speed.  VectorE does pairwise-max
and min reductions.  The grid is sharded 16 blocks (16x8 px) per core with
rank-matched assignment (cores get one block per size class, so the shared
SPMD program's per-slot shapes track the distribution, not the max); per-block
candidate lists are culled by a mathematically safe distance cut.
"""

import math
import numpy as np

GRID = 128
GAMMA = 200.0
DELTA = 2.0 / (GRID - 1)
BY, BX = 16, 8                  # block = 16 rows x 8 cols of pixels
NBY, NBX = GRID // BY, GRID // BX
NBLK = NBY * NBX                # 128 blocks
NCORES = 8
BPC = NBLK // NCORES            # 16 blocks per core
BIG = 1.0e6                     # padding / "dropped" distance^2
SLACK = math.log(1e5) / GAMMA   # exp slack for culling (rel err <= 1e-5)
PQUANT = 32                     # pair-count padding quantum
SQUANT = 64                     # single-count padding quantum

_compiled_cache = {}


# ----------------------------------------------------------------------------
# host-side geometry / coefficient construction
# ----------------------------------------------------------------------------

def _trunc12(x):
    """Round float32 array to 12 explicit mantissa bits (fp32r-exact)."""
    x = np.asarray(x, np.float64)
    m, e = np.frexp(x)
    return np.ldexp(np.round(m * 4096.0) / 4096.0, e).astype(np.float32)


def _block_geom():
    geoms = []
    for b in range(NBLK):
        brow, bcol = b // NBX, b % NBX
        X0 = (bcol * BX) * DELTA - 1.0
        Y0 = (brow * BY) * DELTA - 1.0
        # 4x4 sub-sample centers (4x2 px sub-blocks) + covering radius
        sxs = [X0 + (sx * 2 + 0.5) * DELTA for sx in range(BX // 2)]
        sys_ = [Y0 + (sy * 4 + 1.5) * DELTA for sy in range(BY // 4)]
        samples = [(sx, sy) for sy in sys_ for sx in sxs]
        hsub = math.hypot(0.5 * DELTA, 1.5 * DELTA)
        cx = X0 + (BX - 1) / 2.0 * DELTA
        cy = Y0 + (BY - 1) / 2.0 * DELTA
        hb = math.hypot((BX - 1) / 2.0 * DELTA, (BY - 1) / 2.0 * DELTA)
        geoms.append((X0, Y0, cx, cy, hb, samples, hsub))
    return geoms


_GEOMS = _block_geom()


def _features():
    """lhsT features [12, 128]: rows [F6; F6], F6 = [dx2, dxdy, dy2, dx, dy, 1]."""
    dx = np.arange(BX, dtype=np.float64)
    dy = np.arange(BY, dtype=np.float64)
    DXg, DYg = np.meshgrid(dx, dy)
    dxf = DXg.reshape(-1)                      # p = iy*BX + ix
    dyf = DYg.reshape(-1)
    F6 = np.stack([dxf * dxf, dxf * dyf, dyf * dyf, dxf, dyf,
                   np.ones_like(dxf)], axis=0)
    return np.concatenate([F6, F6], axis=0).astype(np.float32)  # [12, 128]


def _local_coeffs(quads, X0, Y0):
    """[n, 6] f64 quadratics over real coords -> [12, n] f32 hi/lo local rows."""
    a, b, c, d, e, f = (quads[:, i] for i in range(6))
    A2 = a * DELTA * DELTA
    B2 = b * DELTA * DELTA
    C2 = c * DELTA * DELTA
    D1 = (2 * a * X0 + b * Y0 + d) * DELTA
    E1 = (2 * c * Y0 + b * X0 + e) * DELTA
    F0 = a * X0 * X0 + b * X0 * Y0 + c * Y0 * Y0 + d * X0 + e * Y0 + f
    q = np.stack([A2, B2, C2, D1, E1, F0], axis=0)
    hi = _trunc12(q)
    lo = (q - hi.astype(np.float64)).astype(np.float32)
    return np.concatenate([hi, lo], axis=0)


def _transform_geometry(coords, is_pred):
    coords = np.asarray(coords, np.float32)
    kps = ((coords[:, :2] - np.float32(0.5)) * np.float32(2.0)).astype(np.float64)
    mask = (coords[:, 2] > 0.5) if is_pred else (coords[:, 2] != 0.0)
    keep = ~mask[:-1]
    A, B = kps[:-1], kps[1:]
    c = (A + B) / 2
    hv = (A - B) / 2
    r = np.hypot(hv[:, 0], hv[:, 1])
    rs = np.where(r > 0, r, 1)
    ux = np.where(r > 0, hv[:, 0] / rs, 1.0)
    uy = np.where(r > 0, hv[:, 1] / rs, 0.0)
    ep_act = np.zeros(len(kps), bool)
    ep_act[:-1] |= keep
    ep_act[1:] |= keep
    return dict(kps=kps, keep=keep, A=A, B=B, c=c, r=r,
                ux=ux, uy=uy, nx=-uy, ny=ux, ep_act=ep_act)


def _seg_point_dists(pts, geo):
    """pts [m, 2] -> distances [m, S] to all segments (f64)."""
    A, B = geo["A"], geo["B"]
    ab = B - A
    den = (ab * ab).sum(1)
    dens = np.where(den > 0, den, 1)
    t = ((pts[:, None, :] - A[None]) * ab[None]).sum(-1) / dens[None]
    t = np.clip(np.where(den[None] > 0, t, 0.0), 0.0, 1.0)
    proj = A[None] + t[..., None] * ab[None]
    dd = pts[:, None, :] - proj
    return np.hypot(dd[..., 0], dd[..., 1])


def _build_block_lists(geo, block):
    """Candidates for one (transform, block).

    Returns (pair_quads [np_, 2, 6], single_quads [ns, 6]) f64.
    """
    X0, Y0, cx, cy, hb, samples, hsub = _GEOMS[block]
    keep = geo["keep"]
    if not keep.any():
        return np.zeros((0, 2, 6)), np.zeros((0, 6))
    pts = np.asarray(samples)                   # [m, 2]
    dmat = _seg_point_dists(pts, geo)           # [m, S]
    dact = np.where(keep[None], dmat, np.inf)
    Dm = dact.min(1)                            # per-sample nearest active dist
    Rm = np.sqrt((Dm + hsub) ** 2 + SLACK) + hsub   # per-sample keep radius
    kept = keep & (dmat <= Rm[:, None]).any(0)

    c, r = geo["c"], geo["r"]
    # per-sample axis coordinate m_i for each segment: [m, S]
    mS = ((pts[:, None, 0] - c[None, :, 0]) * geo["ux"][None]
          + (pts[:, None, 1] - c[None, :, 1]) * geo["uy"][None])
    inside = (np.abs(mS) <= (r - hsub)[None]).all(0)
    outside = ((mS >= (r + hsub)[None]).all(0)
               | (mS <= -(r + hsub)[None]).all(0))
    pair_sel = kept & ~inside & ~outside
    singleQ_sel = kept & inside
    # cap-side reachability (for endpoint wedge culling)
    reachA = (mS >= (r - hsub)[None]).any(0)    # block reaches beyond A end
    reachB = (mS <= -(r - hsub)[None]).any(0)   # ... beyond B end

    def q_perp(idx):
        nx, ny = geo["nx"][idx], geo["ny"][idx]
        cxs, cys = c[idx, 0], c[idx, 1]
        c0 = -(nx * cxs + ny * cys)
        return np.stack([nx * nx, 2 * nx * ny, ny * ny,
                         2 * nx * c0, 2 * ny * c0, c0 * c0], axis=1)

    def q_circ(px, py, rr2):
        one = np.ones_like(px)
        return np.stack([one, 0 * one, one, -2 * px, -2 * py,
                         px * px + py * py - rr2], axis=1)

    idx_p = np.nonzero(pair_sel)[0]
    pair_quads = np.zeros((len(idx_p), 2, 6))
    if len(idx_p):
        pair_quads[:, 0, :] = q_perp(idx_p)
        pair_quads[:, 1, :] = q_circ(c[idx_p, 0], c[idx_p, 1], r[idx_p] ** 2)

    idx_s = np.nonzero(singleQ_sel)[0]
    singles = [q_perp(idx_s)] if len(idx_s) else []

    kps = geo["kps"]
    # endpoint kps[i] is the A-end of segment i and the B-end of segment i-1;
    # it is only needed where the block reaches beyond that cap.
    npnt = len(kps)
    wedge = np.zeros(npnt, bool)
    wedge[:-1] |= kept & reachA                 # as A-end of segment i
    wedge[1:] |= kept & reachB                  # as B-end of segment i-1
    dE = np.hypot(kps[:, None, 0] - pts[None, :, 0],
                  kps[:, None, 1] - pts[None, :, 1])   # [P, m]
    ep_sel = wedge & (dE <= Rm[None, :]).any(1)
    idx_e = np.nonzero(ep_sel)[0]
    if len(idx_e):
        singles.append(q_circ(kps[idx_e, 0], kps[idx_e, 1], np.zeros(len(idx_e))))
    single_quads = np.concatenate(singles, axis=0) if singles else np.zeros((0, 6))
    return pair_quads, single_quads


def _roundup(x, q):
    return max(q, ((x + q - 1) // q) * q)


NSLOTS = 2 * BPC                # 32 (block, transform) work items per core


def build_tables(pred_coords, gt_coords):
    """Build the execution plan + per-core coefficient tables.

    Work items are (block, transform) pairs, sharded 32 per core with
    rank-matched sizes.  Returns (coef [NCORES, 12, C_total], plan):
      plan["items"][cidx][slot] = (block, transform)
      plan["key"][slot] = (NP, NS); plan["offs"][slot] = column offset.
    """
    geos = [_transform_geometry(gt_coords, False),
            _transform_geometry(pred_coords, True)]
    lists = []
    meta = []
    for b in range(NBLK):
        for t in range(2):
            pq, sq = _build_block_lists(geos[t], b)
            lists.append((pq, sq))
            meta.append((b, t))
    np_ns = np.array([[len(pq), len(sq)] for pq, sq in lists])

    # sort items by singles count desc, then rebalance pair counts within
    # 4-rank-group windows so per-slot caps track the distribution
    order = np.argsort(-np_ns[:, 1]).copy()
    for g0 in range(0, NSLOTS, 4):
        seg = order[g0 * NCORES:(g0 + 4) * NCORES]
        seg = seg[np.argsort(-np_ns[seg, 0])]
        order[g0 * NCORES:(g0 + 4) * NCORES] = seg

    items = [[None] * NSLOTS for _ in range(NCORES)]
    key = []
    offs = [0]
    for s in range(NSLOTS):
        grp = order[s * NCORES:(s + 1) * NCORES]
        NP = _roundup(int(np_ns[grp, 0].max()), PQUANT)
        NS = _roundup(int(np_ns[grp, 1].max()), SQUANT)
        key.append((NP, NS))
        offs.append(offs[-1] + 2 * NP + NS)
        for cidx in range(NCORES):
            items[cidx][s] = meta[grp[cidx]]
    C_total = offs[-1]

    coef = np.zeros((NCORES, 12, C_total), np.float32)
    coef[:, 5, :] = BIG                         # default pad: const hi = BIG
    for s in range(NSLOTS):
        NP, NS = key[s]
        grp = order[s * NCORES:(s + 1) * NCORES]
        for cidx in range(NCORES):
            idx = grp[cidx]
            b, t = meta[idx]
            pq, sq = lists[idx]
            X0, Y0 = _GEOMS[b][0], _GEOMS[b][1]
            quads = np.zeros((2 * NP + NS, 6))
            quads[:, 5] = BIG
            if len(pq):
                quads[:len(pq)] = pq[:, 0]      # [Q cols | Q2 cols]
                quads[NP:NP + len(pq)] = pq[:, 1]
            if len(sq):
                quads[2 * NP:2 * NP + len(sq)] = sq
            coef[cidx, :, offs[s]:offs[s] + 2 * NP + NS] = \
                _local_coeffs(quads, X0, Y0)
    plan = dict(items=items, key=tuple(key), offs=offs, C_total=C_total)
    return coef, plan


# ----------------------------------------------------------------------------
# bass kernel build
# ----------------------------------------------------------------------------

def build_kernel(key, C_total, repeat=1):
    """key: per-slot (NP0, NS0, NP1, NS1) tuples; sizes baked statically."""
    import concourse.bacc as bacc
    import concourse.mybir as mybir
    import concourse.tile as tile

    f32, f32r = mybir.dt.float32, mybir.dt.float32r
    nc = bacc.Bacc(None, target_bir_lowering=False)
    feat_d = nc.dram_tensor("feat", [12, 128], f32, kind="ExternalInput")
    coef_d = nc.dram_tensor("coef", [12, C_total], f32, kind="ExternalInput")
    out_d = nc.dram_tensor("out", [128, NSLOTS], f32, kind="ExternalOutput")

    maxscr = 1024

    with tile.TileContext(nc) as tc:
        with (
            tc.tile_pool(name="feat", bufs=1) as featp,
            tc.tile_pool(name="coef", bufs=2) as coefp,
            tc.tile_pool(name="outsb", bufs=1) as outp,
            tc.tile_pool(name="scr", bufs=2) as scrp,
            tc.tile_pool(name="cpy", bufs=3) as cpyp,
            tc.tile_pool(name="acc", bufs=3) as accp,
            tc.tile_pool(name="ppsum", bufs=2, space="PSUM") as ppsum,
            tc.tile_pool(name="spsum", bufs=3, space="PSUM") as spsum,
        ):
            feat = featp.tile([12, 128], f32r)
            nc.gpsimd.dma_start(feat[:], feat_d[:].bitcast(f32r))
            outsb = outp.tile([128, NSLOTS], f32)

            def mm_fill(ptile, cf, cf_off, ncols):
                for o in range(0, ncols, 512):
                    n = min(512, ncols - o)
                    nc.tensor.matmul(ptile[:, o:o + n], feat[:],
                                     cf[:, cf_off + o:cf_off + o + n],
                                     start=True, stop=True)

            def body(_iv=None):
                offs = [0]
                for (NP, NS) in key:
                    offs.append(offs[-1] + 2 * NP + NS)
                for s, (NP, NS) in enumerate(key):
                    cf = coefp.tile([12, offs[s + 1] - offs[s]], f32r, tag="cf")
                    nc.gpsimd.dma_start(
                        cf[:], coef_d[:, offs[s]:offs[s + 1]].bitcast(f32r))
                    u_s = (NS + 1023) // 1024
                    parts = scrp.tile([128, maxscr], f32, tag="parts")
                    # pairs [Q | Q2]: ScalarE bounces Q2 PSUM->SBUF, DVE
                    # computes max(Q, Q2copy) straight into parts
                    for pc in range(0, NP, 512):
                        npair = min(512, NP - pc)
                        ptA = ppsum.tile([128, 512], f32, tag="pp")
                        ptB = ppsum.tile([128, 512], f32, tag="pp")
                        mm_fill(ptA, cf, pc, npair)
                        mm_fill(ptB, cf, NP + pc, npair)
                        cb = cpyp.tile([128, 512], f32, tag="cpy")
                        nc.scalar.copy(cb[:, 0:npair], ptB[:, 0:npair])
                        nc.vector.tensor_tensor(
                            parts[:, pc:pc + npair], ptA[:, 0:npair],
                            cb[:, 0:npair], op=mybir.AluOpType.max)
                    # singles: reduce-min straight from PSUM into parts
                    for j in range(u_s):
                        ncols = min(1024, NS - j * 1024)
                        st = spsum.tile([128, 1024], f32, tag="sp")
                        mm_fill(st, cf, 2 * NP + j * 1024, ncols)
                        nc.vector.tensor_reduce(
                            parts[:, NP + j:NP + j + 1], st[:, 0:ncols],
                            axis=mybir.AxisListType.X, op=mybir.AluOpType.min)
                    nc.vector.tensor_reduce(
                        outsb[:, s:s + 1], parts[:, 0:NP + u_s],
                        axis=mybir.AxisListType.X, op=mybir.AluOpType.min)

            if repeat == 1:
                body()
            else:
                with tc.For_i(0, repeat, 1) as iv:
                    body(iv)
            nc.gpsimd.dma_start(out_d[:], outsb[:])
    nc.compile()
    return nc


def get_runner(key, C_total, repeat=1):
    ck = (key, C_total, repeat)
    if ck not in _compiled_cache:
        nc = build_kernel(key, C_total, repeat)
        _compiled_cache[ck] = _SpmdRunner(nc, NCORES)
    return _compiled_cache[ck]


# ----------------------------------------------------------------------------
# jit-once SPMD runner (axon PJRT path)
# ----------------------------------------------------------------------------

class _SpmdRunner:
    def __init__(self, nc, n_cores):
        import jax
        import concourse.mybir as mybir
        from jax.sharding import Mesh, PartitionSpec
        from jax.experimental.shard_map import shard_map
        from concourse.bass2jax import (_bass_exec_p, install_neuronx_cc_hook,
                                        partition_id_tensor)
        self.jax = jax
        install_neuronx_cc_hook()
        self.nc = nc
        self.n_cores = n_cores
        partition_name = (nc.partition_id_tensor.name
                          if nc.partition_id_tensor else None)
        in_names, out_names, out_avals, zero_outs = [], [], [], []
        for alloc in nc.m.functions[0].allocations:
            if not isinstance(alloc, mybir.MemoryLocationSet):
                continue
            name = alloc.memorylocations[0].name
            if alloc.kind == "ExternalInput":
                if name != partition_name:
                    in_names.append(name)
            elif alloc.kind == "ExternalOutput":
                out_names.append(name)
                shape = tuple(alloc.tensor_shape)
                dtype = mybir.dt.np(alloc.dtype)
                out_avals.append(jax.core.ShapedArray(shape, dtype))
                zero_outs.append(np.zeros(shape, dtype))
        self.in_names = in_names
        self.out_names = out_names
        self.zero_outs = zero_outs
        n_params, n_outs = len(in_names), len(out_names)
        all_in = in_names + out_names + ([partition_name] if partition_name else [])

        def _body(*args):
            operands = list(args)
            if partition_name is not None:
                operands.append(partition_id_tensor())
            outs = _bass_exec_p.bind(
                *operands, out_avals=tuple(out_avals), in_names=tuple(all_in),
                out_names=tuple(out_names), lowering_input_output_aliases=(),
                sim_require_finite=True, sim_require_nnan=True, nc=nc)
            return tuple(outs)

        devices = jax.devices()[:n_cores]
        self.mesh = Mesh(np.asarray(devices), ("core",))
        self.fn = jax.jit(
            shard_map(_body, mesh=self.mesh,
                      in_specs=(PartitionSpec("core"),) * (n_params + n_outs),
                      out_specs=(PartitionSpec("core"),) * n_outs,
                      check_rep=False),
            donate_argnums=tuple(range(n_params, n_params + n_outs)),
            keep_unused=True)
        self.sharding = jax.sharding.NamedSharding(self.mesh, PartitionSpec("core"))

    def put_inputs(self, in_maps):
        return [self.jax.device_put(
                    np.concatenate([np.asarray(m[n]) for m in in_maps], axis=0),
                    self.sharding)
                for n in self.in_names]

    def run(self, dev_in):
        zo = [self.jax.device_put(np.concatenate([z] * self.n_cores, axis=0),
                                  self.sharding) for z in self.zero_outs]
        outs = self.fn(*dev_in, *zo)
        self.jax.block_until_ready(outs)
        results = []
        for c in range(self.n_cores):
            m = {}
            for i, name in enumerate(self.out_names):
                arr = np.asarray(outs[i])
                per = arr.shape[0] // self.n_cores
                m[name] = arr[c * per:(c + 1) * per]
            results.append(m)
        return results


# ----------------------------------------------------------------------------
# entry point
# ----------------------------------------------------------------------------

def _finish(d2_gt, d2_pred):
    beta_g = np.exp(-GAMMA * d2_gt.astype(np.float64))
    beta_p = np.exp(-GAMMA * d2_pred.astype(np.float64))
    return np.array(np.mean((beta_p - beta_g) ** 2), dtype=np.float32)


def _assemble(results, plan):
    d2 = np.zeros((2, GRID, GRID), np.float32)
    for cidx in range(NCORES):
        out = results[cidx]["out"]          # [128, NSLOTS]
        for s in range(NSLOTS):
            b, t = plan["items"][cidx][s]
            brow, bcol = b // NBX, b % NBX
            d2[t, brow * BY:(brow + 1) * BY,
               bcol * BX:(bcol + 1) * BX] = out[:, s].reshape(BY, BX)
    return d2


def kernel(pred_coords, gt_coords):
    coef, plan = build_tables(pred_coords, gt_coords)
    feat = _features()
    runner = get_runner(plan["key"], plan["C_total"])
    in_maps = [{"feat": feat, "coef": coef[c]} for c in range(NCORES)]
    dev_in = runner.put_inputs(in_maps)
    results = runner.run(dev_in)
    d2 = _assemble(results, plan)
    return _finish(d2[0], d2[1])
